# revision 89
# speedup vs baseline: 1.0541x; 1.0012x over previous
"""Trainium2 Bass kernel for nn_BottleneckFusion (STCN memory readout + ResBlock
+ CBAM + PSP + bottleneck), 8-core SPMD.

Sharding: core c -> (batch b = c//2, half h = c%2).
  Phase A (attention): TM split across the pair (4 memory frames each); the
    affinity/exp/value loop is software-pipelined (affinity matmuls for slice
    i+1 issue before slice i's value matmuls) and the sumexp accumulates on
    DVE+GpSimd; the flash combine of (unnormalized value, sumexp) is a
    pairwise ReduceScatter that delivers each core exactly its clamped 21-row
    x-window, pre-summed (payload 345KB vs 1MB for the old AllGather).
  Phase B (convs/CBAM/PSP): row-half split with halo recompute. The
    query-side halves of the rb1/rbd convs run under the ReduceScatter in 6
    held PSUM banks; the xb own-row block finishes first so the channel-gate
    stats AllGather (per-channel sum via the activation's accum_out) flies
    while the halo rows convolve; the comp path works on gated xc directly
    (per-partition scale, PE transposes, free-dim maxes, scaled-ones-matmul
    mean); sig broadcasts via gpsimd partition_broadcast; the PSP pool
    AllGather (bf16) overlaps the fused-input chunks of the bottleneck conv,
    and the 1x1 pool convs produce pd directly in transposed (cell-major)
    layout so the upsample needs no PE transposes.

kernel(**inputs) takes the FULL unsharded inputs and returns the FULL output.
"""
import sys

sys.path.insert(0, "/opt/trn_rl_repo")

import numpy as np
import ml_dtypes

import concourse.bass as bass
import concourse.bacc as bacc
import concourse.mybir as mybir
import concourse.tile as tile
from concourse.bass_utils import run_bass_kernel_spmd

BF16 = ml_dtypes.bfloat16
F16 = np.float16
bf = mybir.dt.bfloat16
f16 = mybir.dt.float16
f32 = mybir.dt.float32
AF = mybir.ActivationFunctionType
ALU = mybir.AluOpType
AX = mybir.AxisListType

N_CORES = 8
B, TM, CIN, CK, CV, COUT, H, W = 4, 8, 256, 64, 256, 256, 32, 32
EPS = 1e-5

# local row coordinates: l = image_row - (r0 - 5), l in 0..25
XROWS = 26                 # x window rows (image r0-5 .. r0+20)
CROWS = 22                 # xb/xc/comp local rows (image r0-3 .. r0+18)
PIXPAD = 768               # padded xc free size (22*34=748 -> 768)
PAIRS = [[0, 1], [2, 3], [4, 5], [6, 7]]
UPS = (2, 4, 8)            # upsampled PSP scales
# full pool pyramid offsets [s1, s2, s4, s8] and own-partial offsets
FOFF = {1: 0, 2: 1, 4: 5, 8: 21}
POFF = {1: 0, 2: 1, 4: 3, 8: 11}


def interp_matrix(s_in, s_out=32):
    if s_in == 1:
        return np.ones((s_out, 1), np.float32)
    c = np.arange(s_out) * (s_in - 1) / (s_out - 1)
    lo = np.floor(c).astype(np.int64)
    hi = np.minimum(lo + 1, s_in - 1)
    w = (c - lo).astype(np.float32)
    M = np.zeros((s_out, s_in), np.float32)
    M[np.arange(s_out), lo] += 1.0 - w
    M[np.arange(s_out), hi] += w
    return M


# ---------------------------------------------------------------------------
# Host-side input preparation
# ---------------------------------------------------------------------------

def _pad_hw(a):
    out = np.zeros(a.shape[:-2] + (34, 34), a.dtype)
    out[..., 1:33, 1:33] = a
    return out


def _chw_chunks(a):
    """[256, ...] -> [128, 2, ...] (partition, chunk)."""
    return a.reshape(2, 128, *a.shape[1:]).transpose(
        1, 0, *range(2, a.ndim + 1))


def prep_core_inputs(inputs, core):
    b, h = core // 2, core % 2
    r0 = 16 * h
    g = {}

    f16_q = np.asarray(inputs["f16_q"], np.float32)
    f16_m = np.asarray(inputs["f16_m"], np.float32)
    value_m = np.asarray(inputs["value_m"], np.float32)

    # xm: [128, 2, 4, 34, 34] padded memory frames
    src = f16_m[b, 4 * h: 4 * h + 4]                        # [4, 256, 32, 32]
    src = src.reshape(4, 2, 128, 32, 32).transpose(2, 1, 0, 3, 4)
    g["xm"] = _pad_hw(src).astype(F16)

    # xq: [128, 2, 34, 34] padded query
    q = _chw_chunks(f16_q[b, 0])                            # [128, 2, 32, 32]
    g["xq"] = _pad_hw(q).astype(F16)

    # vT: [128, 32, 256] transposed value
    V = value_m[b][:, 4 * h: 4 * h + 4].reshape(CV, 4096)
    g["vT"] = np.ascontiguousarray(
        V.T.reshape(32, 128, CV).transpose(1, 0, 2)).astype(BF16)  # stays bf16 (matches e)

    # x window q-part: [128, 2, 26, 34]
    qw = np.zeros((128, 2, XROWS, 34), np.float32)
    for l in range(XROWS):
        img = r0 - 5 + l
        if 0 <= img <= 31:
            qw[:, :, l, 1:33] = q[:, :, img, :]
    g["xqb_raw"] = qw.astype(F16)
    g["xqb_relu"] = np.maximum(qw, 0.0).astype(F16)

    pk_w = np.asarray(inputs["pk_w"], np.float32)
    g["pk_wT"] = np.ascontiguousarray(
        pk_w.reshape(CK, 2, 128, 3, 3).transpose(2, 1, 3, 4, 0)).astype(F16)
    pk_b = np.asarray(inputs["pk_b"], np.float32)
    g["pkb2"] = np.concatenate([pk_b, pk_b]).reshape(128, 1).astype(np.float32)

    def conv_lhsT(w, kc):
        co = w.shape[0]
        return np.ascontiguousarray(
            w.reshape(co, kc, 128, 3, 3).transpose(2, 1, 3, 4, 0)).astype(F16)

    g["rb1_wT"] = conv_lhsT(np.asarray(inputs["rb1_w"], np.float32), 4)
    g["rb2_wT"] = conv_lhsT(np.asarray(inputs["rb2_w"], np.float32), 2)
    g["rbd_wT"] = conv_lhsT(np.asarray(inputs["rbd_w"], np.float32), 4)
    g["rb1_b"] = np.asarray(inputs["rb1_b"], np.float32).reshape(2, 128).T.copy()
    g["xb_bias"] = (np.asarray(inputs["rb2_b"], np.float32)
                    + np.asarray(inputs["rbd_b"], np.float32)
                    ).reshape(2, 128).T.copy()

    w1 = np.asarray(inputs["mlp_w1"], np.float32)           # [16, 256]
    g["mlp_w1T"] = np.ascontiguousarray(
        w1.reshape(16, 2, 128).transpose(2, 1, 0)).copy()   # [128, 2, 16]
    g["mlp_w1Ts"] = (g["mlp_w1T"] / 1024.0).copy()          # folds the mean
    g["mlp_b1"] = np.asarray(inputs["mlp_b1"], np.float32).reshape(16, 1).copy()
    g["mlp_w2T"] = np.ascontiguousarray(
        np.asarray(inputs["mlp_w2"], np.float32).T).copy()  # [16, 256]
    g["mlp_b2x2"] = (2.0 * np.asarray(inputs["mlp_b2"], np.float32)
                     ).reshape(2, 128).T.copy()

    spw = np.asarray(inputs["sp_w"], np.float32)[0]       # [2, 7, 7]
    g["spw_r"] = np.ascontiguousarray(
        spw.reshape(14, 7)).astype(np.float16)                # [(ch,dy), dx]
    bn_scale = float(np.asarray(inputs["sp_g"], np.float32)[0]) / float(
        np.sqrt(1.0 + EPS))
    bn_bias = float(np.asarray(inputs["sp_b"], np.float32)[0])
    g["bn_sb"] = np.array([[bn_scale, bn_bias]], np.float32)

    maskT = np.zeros((128, 6, 1), np.float16)
    mask_mean = np.zeros((1, 768), np.float16)
    for pix in range(CROWS * 34):
        img = r0 - 3 + pix // 34
        if 0 <= img <= 31:
            maskT[pix % 128, pix // 128, 0] = 1.0
            mask_mean[0, pix] = 1.0
    g["comp_maskT"] = maskT
    g["mask_mean"] = mask_mean

    pw = np.zeros((128, 2, 4, 64), np.float32)
    for si, s in enumerate((1, 2, 4, 8)):
        wc = np.asarray(inputs[f"psp_w{s}"], np.float32)[:, :, 0, 0]
        scale = 1.0 / ((32 // s) ** 2)
        pw[:, :, si, :] = (wc.T * scale).reshape(2, 128, 64).transpose(1, 0, 2)
    g["psp_wT"] = pw.astype(BF16)

    # folded upsample operators: Wup[row(k), si, (r*32+c)] = M[r0+r, jr] *
    # M[c, jc] where k = jr*s+jc. Rows are gap-laid-out to match the
    # transposed-pd psum layout (slot-1 cells land at partition 32+).
    ROWMAP = {2: lambda k: k if k < 2 else 30 + k,
              4: lambda k: k if k < 8 else 24 + k,
              8: lambda k: k}
    Wup = np.zeros((64, 3, 512), np.float32)
    for si, s in enumerate(UPS):
        M = interp_matrix(s)
        Mrr = M[r0: r0 + 16, :]                 # [16, s]
        for jr in range(s):
            for jc in range(s):
                Wup[ROWMAP[s](jr * s + jc), si, :] = np.outer(
                    Mrr[:, jr], M[:, jc]).reshape(512)
    g["Wup"] = Wup.astype(BF16)

    bott_w = np.asarray(inputs["bott_w"], np.float32)[:, :, 0, 0]
    g["bott_wT"] = np.ascontiguousarray(
        bott_w.reshape(COUT, 4, 128).transpose(2, 1, 0)).astype(F16)
    g["bott_b"] = np.asarray(inputs["bott_b"], np.float32).reshape(2, 128).T.copy()

    rmask = np.zeros((1, XROWS, 34), np.float16)
    for l in range(XROWS):
        if 0 <= r0 - 5 + l <= 31:
            rmask[0, l, :] = 1.0
    g["rmask"] = rmask

    g["ident"] = np.eye(128, dtype=np.float32)
    g["ident_h"] = np.eye(128, dtype=np.float16)
    return g


INPUT_SPECS = [
    ("xm", [128, 2, 4, 34, 34], f16),
    ("xq", [128, 2, 34, 34], f16),
    ("vT", [128, 32, 256], bf),
    ("xqb_raw", [128, 2, XROWS, 34], f16),
    ("xqb_relu", [128, 2, XROWS, 34], f16),
    ("pk_wT", [128, 2, 3, 3, 64], f16),
    ("pkb2", [128, 1], f32),
    ("rb1_wT", [128, 4, 3, 3, 256], f16),
    ("rb2_wT", [128, 2, 3, 3, 256], f16),
    ("rbd_wT", [128, 4, 3, 3, 256], f16),
    ("rb1_b", [128, 2], f32),
    ("xb_bias", [128, 2], f32),
    ("mlp_w1T", [128, 2, 16], f32),
    ("mlp_w1Ts", [128, 2, 16], f32),
    ("mlp_b1", [16, 1], f32),
    ("mlp_w2T", [16, 256], f32),
    ("mlp_b2x2", [128, 2], f32),
    ("spw_r", [14, 7], f16),
    ("bn_sb", [1, 2], f32),
    ("comp_maskT", [128, 6, 1], f16),
    ("mask_mean", [1, 768], f16),
    ("psp_wT", [128, 2, 4, 64], bf),
    ("Wup", [64, 3, 512], bf),
    ("bott_wT", [128, 4, 256], f16),
    ("bott_b", [128, 2], f32),
    ("ident", [128, 128], f32),
    ("ident_h", [128, 128], f16),
    ("rmask", [1, XROWS, 34], f16),
]


# ---------------------------------------------------------------------------
# Device kernel
# ---------------------------------------------------------------------------

def build(stage="full"):
    nc = bacc.Bacc("TRN2", target_bir_lowering=False, debug=False,
                   num_devices=N_CORES)
    prm = {n: nc.declare_dram_parameter(n, sh, dt, isOutput=False)
           for n, sh, dt in INPUT_SPECS}
    if stage == "A":
        out_prm = nc.declare_dram_parameter("out_a", [257, 1024], f32,
                                            isOutput=True)
    else:
        out_prm = nc.declare_dram_parameter("out", [128, 2, 16, 32], f16,
                                            isOutput=True)
    if stage == "dbg":
        for n, sh, dt in [("dbg_xraw", [128, 4, XROWS, 34], f16),
                          ("dbg_xb", [128, 2, CROWS, 34], f32),
                          ("dbg_gate", [128, 2, 1], f32),
                          ("dbg_sig", [1, 512], bf),
                          ("dbg_fused", [128, 2, 16, 32], f16),
                          ("dbg_pd", [64, 85], f32),
                          ("dbg_pri0", [128, 512], f16),
                          ("dbg_pri1", [128, 512], f16)]:
            prm[n] = nc.declare_dram_parameter(n, sh, dt, isOutput=True)
    with tile.TileContext(nc) as tc:
        _emit(tc, nc, prm, stage, out_prm)
    nc.compile()
    return nc


def _emit(tc, nc, prm, stage, out_prm):
    import contextlib
    es = contextlib.ExitStack()
    with es:
        wpool = es.enter_context(tc.tile_pool(name="wpool", bufs=1))
        apool = es.enter_context(tc.tile_pool(name="apool", bufs=1))
        dram = es.enter_context(tc.tile_pool(name="dram", bufs=1, space="DRAM"))
        aonly_cm = tc.tile_pool(name="aonly", bufs=1)
        aonly = aonly_cm.__enter__()

        def load(name, pool=wpool):
            t = pool.tile(list(prm[name].shape), prm[name].dtype,
                          name=f"{name}_sb")
            nc.sync.dma_start(t[:], prm[name][:])
            return t

        pk_wT = wpool.tile([128, 2, 3, 3, 64], f16, name="pk_wT_sb")
        for j in range(2):
            for dy in range(3):
                nc.sync.dma_start(pk_wT[:, j, dy], prm["pk_wT"][:, j, dy])
        pkb2 = load("pkb2")
        xq_sb = aonly.tile([128, 2, 34, 34], f16, name="xq_sb")
        for j in range(2):
            nc.gpsimd.dma_start(xq_sb[:, j], prm["xq"][:, j])
        xm_sb = aonly.tile([128, 2, 4, 34, 34], f16, name="xm_sb")
        for t in range(4):
            nc.sync.dma_start(xm_sb[:, :, t, :, :], prm["xm"][:, :, t, :, :])
        vT_sb = load("vT", aonly)

        # phase-B weights: load early (DMA bandwidth is free during phase A,
        # and apool space is disjoint from the phase-A-only arena)
        rb1_wT = load("rb1_wT", apool)
        rb2_wT = load("rb2_wT", apool)
        rbd_wT = load("rbd_wT", apool)
        rb1_b = load("rb1_b", apool)
        xb_bias = load("xb_bias", apool)
        mlp_w1T = load("mlp_w1T", apool)
        mlp_w1Ts = load("mlp_w1Ts", apool)
        mlp_b1 = load("mlp_b1", apool)
        mlp_w2T = load("mlp_w2T", apool)
        mlp_b2x2 = load("mlp_b2x2", apool)
        spw_r = load("spw_r", apool)
        bn_sb = load("bn_sb", apool)
        comp_maskT = load("comp_maskT", apool)
        mask_mean = load("mask_mean", apool)
        psp_wT = load("psp_wT", apool)
        Wup = load("Wup", apool)
        bott_wT = load("bott_wT", apool)
        bott_b = load("bott_b", apool)
        ident = load("ident", apool)
        ident_h = load("ident_h", apool)

        ones_bf = wpool.tile([128, 1], bf)
        nc.vector.memset(ones_bf[:], 1.0)

        # im2col buffer for the 7x7 spatial conv: zero the pad columns early
        il = wpool.tile([14, 16, 38], f16, name="il")
        nc.vector.memset(il[:, :, 0:2], 0.0)
        nc.vector.memset(il[:, :, 36:38], 0.0)

        # ================= phase A =================
        mk_sb = aonly.tile([128, 2, 1024], f16)
        qk_sb = aonly.tile([128, 1024], f16)

        # psAff/psV opened BEFORE psA so their banks are disjoint from the
        # conv psum banks — the affinity/value matmuls then start without a
        # bank-reuse WAR stall on the last mk activation
        psAff_cm = tc.tile_pool(name="psAff", bufs=2, space="PSUM")
        psAff = psAff_cm.__enter__()
        psV_cm = tc.tile_pool(name="psV", bufs=1, space="PSUM")
        psV = psV_cm.__enter__()
        with tc.tile_pool(name="psA", bufs=2, space="PSUM") as psA:
            for n in range(2):
                pq = psA.tile([128, 512], f32, tag="mkps", name="pq")
                k = 0
                for j in range(2):
                    for dy in range(3):
                        for dx in range(3):
                            nc.tensor.matmul(
                                pq[0:64, :], pk_wT[:, j, dy, dx, :],
                                xq_sb[:, j, n * 16 + dy: n * 16 + dy + 16,
                                      dx: dx + 32],
                                start=(k == 0), stop=(k == 17))
                            k += 1
                nc.scalar.activation(
                    qk_sb[0:64, n * 512: (n + 1) * 512], pq[0:64, :],
                    AF.Identity, bias=pkb2[0:64, 0:1])
            # replicate qk to partitions 64..127 so odd-frame mk slices
            # (base partition 64) can stream against it
            nc.sync.dma_start(qk_sb[64:128, :], qk_sb[0:64, :])

            for tp in range(2):
                for n in range(2):
                    pm = psA.tile([128, 512], f32, tag="mkps", name="pm")
                    for par in range(2):
                        t = 2 * tp + par
                        k = 0
                        for j in range(2):
                            for dy in range(3):
                                for dx in range(3):
                                    nc.tensor.matmul(
                                        pm[64 * par: 64 * par + 64, :],
                                        pk_wT[:, j, dy, dx, :],
                                        xm_sb[:, j, t,
                                              n * 16 + dy: n * 16 + dy + 16,
                                              dx: dx + 32],
                                        start=(k == 0), stop=(k == 17),
                                        tile_position=(0, 64 * par),
                                    )
                                    k += 1
                    nc.scalar.activation(
                        mk_sb[:, tp, n * 512: (n + 1) * 512], pm[:, :],
                        AF.Identity, bias=pkb2[:, 0:1])

        # flash-combine via pairwise ReduceScatter: each core receives the
        # summed (value, sumexp) restricted to ITS x-window clamped to the
        # image: half hh covers img rows max(0,16hh-5)..min(31,16hh+20),
        # i.e. 21 rows = px [352*hh : 352*hh + 672].
        arv_in = dram.tile([2, 257, 672], bf)
        arv_out = dram.tile([257, 672], bf)

        if True:
            vps = [psV.tile([128, 1024], f32, name=f"vps{j}") for j in range(2)]
            s_acc = aonly.tile([128, 1024], f32, name="s_acc")
            s_accb = aonly.tile([128, 1024], f32, name="s_accb")

            order = [16 * h + o + 8 * par for h in range(2) for o in range(8)
                     for par in range(2)]

            # software-pipelined: issue slice idx+1's affinity matmuls before
            # slice idx's value matmuls so the PE never waits on the Exp
            def aff_exp(idx):
                i = order[idx]
                t = i >> 3
                pb = i & 7
                tp, par = t >> 1, t & 1
                lhs_aff = mk_sb[64 * par: 64 * par + 64, tp,
                                pb * 128: pb * 128 + 128]
                e_t = aonly.tile([128, 1024], bf, tag="e", name="e_t", bufs=4)
                for qn in range(2):
                    pa = psAff.tile([128, 512], f32, tag="affp", name="pa")
                    nc.tensor.matmul(
                        pa[:, :], lhs_aff,
                        qk_sb[64 * par: 64 * par + 64,
                              qn * 512: (qn + 1) * 512],
                        start=True, stop=True)
                    nc.scalar.activation(
                        e_t[:, qn * 512: (qn + 1) * 512], pa[:, :],
                        AF.Exp, scale=0.125)
                return e_t

            e_cur = aff_exp(0)
            for idx in range(32):
                i = order[idx]
                e_t = e_cur
                if idx < 31:
                    e_cur = aff_exp(idx + 1)
                for j in range(2):
                    for qn in range(2):
                        nc.tensor.matmul(
                            vps[j][:, qn * 512: (qn + 1) * 512],
                            vT_sb[:, i, j * 128: (j + 1) * 128],
                            e_t[:, qn * 512: (qn + 1) * 512],
                            start=(idx == 0), stop=(idx == 31))
                # split the sumexp accumulation DVE/Pool (Pool runs Add at
                # 0.42 efficiency, so give it the smaller share)
                on_pool = idx % 3 == 2
                acc = s_accb if on_pool else s_acc
                eng = nc.gpsimd if on_pool else nc.vector
                if idx == (2 if on_pool else 0):
                    eng.tensor_copy(acc[:, :], e_t[:, :])
                else:
                    eng.tensor_add(acc[:, :], acc[:, :], e_t[:, :])

            v_sb = aonly.tile([128, 2, 1024], bf, name="v_sb")
            s_sb = aonly.tile([1, 1024], bf, name="s_sb")
            nc.vector.tensor_copy(v_sb[:, 0, :], vps[0][:, :])
            nc.scalar.copy(v_sb[:, 1, :], vps[1][:, :])
            # fold the 128-partition sumexp accumulators off the PE:
            # elementwise add on DVE, then a gpsimd partition reduction
            import concourse.bass_isa as bass_isa
            nc.vector.tensor_add(s_acc[:, :], s_acc[:, :], s_accb[:, :])
            s_red = aonly.tile([128, 1024], f32, name="s_red")
            nc.gpsimd.partition_all_reduce(s_red[:, :], s_acc[:, :], 128,
                                           bass_isa.ReduceOp.add)
            nc.vector.tensor_copy(s_sb[:, :], s_red[0:1, :])
            for hh in range(2):
                eng = nc.sync if hh == 0 else nc.scalar
                for j in range(2):
                    eng.dma_start(
                        arv_in[hh, 128 * j: 128 * j + 128, :],
                        v_sb[:, j, 352 * hh: 352 * hh + 672])
                eng.dma_start(arv_in[hh, 256:257, :],
                              s_sb[:, 352 * hh: 352 * hh + 672])

        psV_cm.__exit__(None, None, None)
        psAff_cm.__exit__(None, None, None)

        nc.gpsimd.collective_compute(
            "ReduceScatter", ALU.add, replica_groups=PAIRS,
            ins=[arv_in[:].opt()], outs=[arv_out[:].opt()])

        aonly_cm.__exit__(None, None, None)

        # ================= phase B =================
        wk = es.enter_context(tc.tile_pool(name="wk", bufs=1))

        x_raw = apool.tile([128, 4, XROWS, 34], f16)
        x_relu = apool.tile([128, 4, XROWS, 34], f16)
        for tt in (x_raw, x_relu):
            nc.vector.memset(tt[:, 2:4, :, :], 0.0)
        nc.sync.dma_start(x_raw[:, 0:2, :, :], prm["xqb_raw"][:])
        nc.sync.dma_start(x_relu[:, 0:2, :, :], prm["xqb_relu"][:])

        r1_relu = apool.tile([128, 2, XROWS, 34], f16)
        nc.vector.memset(r1_relu[:, :, 0:1, :], 0.0)
        nc.vector.memset(r1_relu[:, :, 25:26, :], 0.0)
        nc.vector.memset(r1_relu[:, :, :, 0:1], 0.0)
        nc.vector.memset(r1_relu[:, :, :, 33:34], 0.0)
        rmaskb = apool.tile([128, XROWS, 34], f16)
        nc.sync.dma_start(rmaskb[:], prm["rmask"][:].partition_broadcast(128))
        xb = apool.tile([128, 2, PIXPAD], f32)
        xbv = [xb[:, j, 0: CROWS * 34].rearrange("p (r c) -> p r c", c=34)
               for j in range(2)]
        for j in range(2):
            nc.vector.memset(xbv[j][:, :, 0:1], 0.0)
            nc.vector.memset(xbv[j][:, :, 33:34], 0.0)
        nc.vector.memset(xb[:, :, CROWS * 34:], 0.0)

        # conv row blocks: r1 full; xb own rows (xbv 3..18) first so the
        # stats AllGather can launch while the halo rows (0..2, 19..21) are
        # still convolving.
        R1BLK = ((1, 16), (17, 8))
        stats = wk.tile([128, 2, 2], f32, name="stats")
        stats_d = dram.tile([256, 2], f32)
        stats_o = dram.tile([2, 256, 2], f32)

        def conv9(ps, wT, src, m, j, l0, nr, k0, stop_k):
            k = k0
            for dy in range(3):
                for dx in range(3):
                    nc.tensor.matmul(
                        ps[:, : nr * 32],
                        wT[:, j, dy, dx, m * 128: m * 128 + 128],
                        src[:, j, l0 + dy - 1: l0 + dy - 1 + nr, dx: dx + 32],
                        start=(k == 0), stop=(k == stop_k))
                    k += 1
            return k

        psX_cm = tc.tile_pool(name="psX", bufs=1, space="PSUM")
        psX = psX_cm.__enter__()
        psR1_cm = tc.tile_pool(name="psR1", bufs=1, space="PSUM")
        psR1 = psR1_cm.__enter__()
        pr = [[psR1.tile([128, nr * 32], f32, name=f"pr{m}{bi}")
               for bi, (l0, nr) in enumerate(R1BLK)] for m in range(2)]
        pxo = [psX.tile([128, 512], f32, name=f"pxo{m}") for m in range(2)]

        # -- query-side partial convs: run while the ReduceScatter flies
        for m in range(2):
            for bi, (l0, nr) in enumerate(R1BLK):
                k = 0
                for j in range(2):
                    k = conv9(pr[m][bi], rb1_wT, x_relu, m, j, l0, nr, k, -1)
        for m in range(2):
            k = 0
            for j in range(2):
                k = conv9(pxo[m], rbd_wT, x_raw, m, j, 5, 16, k, -1)

        # -- val window lands (RS output), normalize, build x val chunks at
        # the per-core row offset (h=0: x rows 5..25, h=1: x rows 0..20)
        val_win = wk.tile([128, 2, 672], bf, name="val_win")
        s_b = wk.tile([128, 672], bf, name="s_b")
        nc.scalar.dma_start(s_b[:],
                            arv_out[256:257, :].partition_broadcast(128))
        nc.sync.dma_start(val_win[:, 0, :], arv_out[0:128, :])
        nc.scalar.dma_start(val_win[:, 1, :], arv_out[128:256, :])
        # warm the PE (p-state) while the val window is normalized
        with tc.tile_pool(name="psW2", bufs=1, space="PSUM") as psW2:
            pw2 = psW2.tile([1, 128], f32, name="pw2")
            for i in range(48):
                nc.tensor.matmul(pw2[0:1, :], s_b[:, i: i + 1],
                                 s_b[:, 0:128], start=True, stop=True)
        invb = wk.tile([128, 672], f32, name="invb")
        nc.vector.reciprocal(invb[:, :], s_b[:, :])
        voff = (1 - nc.vector.partition_id() % 2) * 5
        voff_a = (1 - nc.scalar.partition_id() % 2) * 5
        for j in range(2):
            vv = val_win[:, j, :].rearrange("p (r c) -> p r c", c=32)
            iv = invb.rearrange("p (r c) -> p r c", c=32)
            nc.vector.tensor_mul(
                x_raw[:, 2 + j, bass.ds(voff, 21), 1:33], vv, iv)
            nc.scalar.activation(
                x_relu[:, 2 + j, bass.ds(voff_a, 21), 1:33],
                x_raw[:, 2 + j, bass.ds(voff_a, 21), 1:33], AF.Relu)

        # -- finish r1 with the val-side chunks
        for m in range(2):
            for bi, (l0, nr) in enumerate(R1BLK):
                k = 18
                for j in (2, 3):
                    k = conv9(pr[m][bi], rb1_wT, x_relu, m, j, l0, nr, k, 35)
                nc.scalar.activation(
                    r1_relu[:, m, l0: l0 + nr, 1:33],
                    pr[m][bi][:, : nr * 32],
                    AF.Relu, bias=rb1_b[:, m: m + 1])
                nc.vector.tensor_mul(r1_relu[:, m, l0: l0 + nr, 1:33],
                                     r1_relu[:, m, l0: l0 + nr, 1:33],
                                     rmaskb[:, l0: l0 + nr, 1:33])
        psR1_cm.__exit__(None, None, None)

        # -- xb own rows: rbd val-side + rb2(r1) -> stats -> AllGather
        # (the per-channel sum falls out of the activation's accum_out)
        for m in range(2):
            k = 18
            for j in (2, 3):
                k = conv9(pxo[m], rbd_wT, x_raw, m, j, 5, 16, k, -1)
            for j in range(2):
                k = conv9(pxo[m], rb2_wT, r1_relu, m, j, 5, 16, k, 53)
            nc.scalar.activation(
                xbv[m][:, 3:19, 1:33], pxo[m][:, :],
                AF.Identity, bias=xb_bias[:, m: m + 1],
                accum_out=stats[:, m, 0:1])
        psX_cm.__exit__(None, None, None)
        for j in range(2):
            nc.vector.tensor_reduce(stats[:, j, 1:2],
                                    xbv[j][:, 3:19, 1:33], AX.XY, ALU.max)
        nc.sync.dma_start(stats_d.rearrange("(j p) k -> p j k", j=2),
                          stats[:, :, :])
        nc.gpsimd.collective_compute(
            "AllGather", ALU.bypass, replica_groups=PAIRS,
            ins=[stats_d[:].opt()], outs=[stats_o[:].opt()])

        # -- halo rows (full conv) run under the stats AllGather, in the
        # banks freed by r1
        with tc.tile_pool(name="psH", bufs=1, space="PSUM") as psH:
            pxh = [[psH.tile([128, 96], f32, name=f"pxh{m}{ci}")
                    for ci in range(2)] for m in range(2)]
            for m in range(2):
                for ci, l0 in enumerate((2, 21)):
                    k = 0
                    for j in range(4):
                        src = x_raw
                        k = conv9(pxh[m][ci], rbd_wT, src, m, j, l0, 3, k, -1)
                    for j in range(2):
                        k = conv9(pxh[m][ci], rb2_wT, r1_relu, m, j, l0, 3,
                                  k, 53)
                    nc.scalar.activation(
                        xbv[m][:, l0 - 2: l0 + 1, 1:33],
                        pxh[m][ci][:, :],
                        AF.Identity, bias=xb_bias[:, m: m + 1])

        wa = wk.tile([128, 3, 1768], f16, name="wa")

        if stage == "dbg":
            nc.sync.dma_start(prm["dbg_xraw"][:], x_raw[:])
            for j in range(2):
                nc.sync.dma_start(prm["dbg_xb"][:, j], xbv[j])

        # ---- CBAM channel gate (stats AllGather result) ----
        sl = wk.tile([128, 2, 2, 2], f32, name="sl")  # [p, slot, j, (sum,max)]
        nc.sync.dma_start(sl[:, :, :, :],
                          stats_o.rearrange("s (j p) k -> p s j k", j=2))
        # mean path rides the matmul accumulation (W1/1024 pre-scaled on the
        # host); only the max path needs a combine op
        gmax = wk.tile([128, 2, 1], f32, name="gmax")
        nc.vector.tensor_max(gmax[:, :, :], sl[:, 0, :, 1:2], sl[:, 1, :, 1:2])

        gate = wk.tile([128, 2, 1], f32, name="gate")
        with tc.tile_pool(name="psG", bufs=1, space="PSUM") as psG:
            ph1 = psG.tile([16, 2], f32, name="ph1")
            k = 0
            for s in range(2):
                for j in range(2):
                    nc.tensor.matmul(ph1[:, 0:1], mlp_w1Ts[:, j, :],
                                     sl[:, s, j, 0:1],
                                     start=(k == 0), stop=(k == 3))
                    k += 1
            for j in range(2):
                nc.tensor.matmul(ph1[:, 1:2], mlp_w1T[:, j, :],
                                 gmax[:, j, :], start=(j == 0), stop=(j == 1))
            h1 = wk.tile([16, 2], f32, name="h1")
            nc.scalar.activation(h1[:, :], ph1[:, :], AF.Relu,
                                 bias=mlp_b1[:, 0:1])
            # W2.relu(h_mean) + W2.relu(h_max) = W2.(relu(h_mean)+relu(h_max))
            hs = wk.tile([16, 1], f32, name="hs")
            nc.vector.tensor_add(hs[:, :], h1[:, 0:1], h1[:, 1:2])
            for j in range(2):
                ph2 = psG.tile([128, 1], f32, tag="ph2", name="ph2")
                nc.tensor.matmul(ph2[:, :], mlp_w2T[:, j * 128: j * 128 + 128],
                                 hs[:, :], start=True, stop=True)
                nc.scalar.activation(gate[:, j, :], ph2[:, :], AF.Sigmoid,
                                     bias=mlp_b2x2[:, j: j + 1])

        if stage == "dbg":
            nc.sync.dma_start(prm["dbg_gate"][:], gate[:])

        # ---- xc = gate * xb (per-partition scale), then pixel-major copies
        xc = apool.tile([128, 2, 768], f16)
        nc.vector.memset(xc[:, :, 748:768], 0.0)
        for (c0, c1) in ((0, 384), (384, 748)):
            nc.scalar.mul(xc[:, 0, c0: c1], xb[:, 0, c0: c1],
                          gate[:, 0, 0:1])
            nc.vector.tensor_scalar(xc[:, 1, c0: c1], xb[:, 1, c0: c1],
                                    gate[:, 1, 0:1], None, ALU.mult)
        xcv = [xc[:, j, 0: CROWS * 34].rearrange("p (r c) -> p r c", c=34)
               for j in range(2)]
        # channel max of xc via gpsimd partition reductions (row-major result
        # lands directly on partition 0 — no PE transposes needed)
        cmx = wk.tile([128, 2, 748], f16, name="cmx")
        for j in range(2):
            nc.gpsimd.partition_all_reduce(cmx[:, j, :], xc[:, j, 0:748],
                                           128, bass_isa.ReduceOp.max)
        comp_max = wk.tile([1, 748], f16, name="comp_max")
        nc.vector.tensor_max(comp_max[:, :], cmx[0:1, 0, :], cmx[0:1, 1, :])
        nc.vector.tensor_mul(comp_max[:, :], comp_max[:, :],
                             mask_mean[:, 0:748])
        mean_sb = wk.tile([1, 748], f16, name="mean_sb")
        onesd = wk.tile([128, 1], f16, name="onesd")
        nc.vector.memset(onesd[:], 1.0 / 256.0)
        with tc.tile_pool(name="psM", bufs=1, space="PSUM") as psM:
            pm1 = psM.tile([1, 748], f32, name="pm1")
            for j in range(2):
                for (o0, nn) in ((0, 512), (512, 236)):
                    nc.tensor.matmul(pm1[0:1, o0: o0 + nn],
                                     onesd[:, 0:1],
                                     xc[:, j, o0: o0 + nn],
                                     start=(j == 0), stop=(j == 1))
            nc.scalar.copy(mean_sb[:, :], pm1[:, :])
        nc.vector.tensor_mul(mean_sb[:, :], mean_sb[:, :],
                             mask_mean[:, 0:748])

        comp_flat = dram.tile([2, 768], f16)
        nc.sync.dma_start(comp_flat[0, 0:748], comp_max[:, :])
        nc.scalar.dma_start(comp_flat[1, 0:748], mean_sb[:, :])
        # gather the 7x7-conv im2col rows straight from comp_flat; the L/R
        # zero-pad columns of il were pre-zeroed at kernel start
        for ch in range(2):
            eng = nc.sync if ch == 0 else nc.scalar
            eng.dma_start(
                il[7 * ch: 7 * ch + 7, :, 2:36],
                bass.AP(comp_flat.tensor, 768 * ch,
                        [[34, 7], [34, 16], [1, 34]]))

        # keep the PE out of its low p-state across the ~5us im2col DMA wait:
        # an Activation-timed ladder gates short dummy matmuls so the tensor
        # engine stays continuously busy until the spatial conv's data lands
        wsrc = x_raw[:, 0:2].rearrange("p j r c -> p (j r c)")
        with tc.tile_pool(name="psW", bufs=1, space="PSUM") as psW:
            pw = psW.tile([1, 128], f32, name="pw")
            for r in range(3):
                src = wsrc if r == 0 else wa[:, r - 1, :]
                nc.scalar.activation(wa[:, r, :], src, AF.Identity)
                for i in range(24):
                    nc.tensor.matmul(pw[0:1, :], wa[:, r, i: i + 1],
                                     wa[:, r, 0:128], start=True, stop=True)
        sig = wk.tile([1, 512], bf, name="sig")
        ones_row = wk.tile([1, 128], bf, name="ones_row")
        nc.vector.memset(ones_row[:], 1.0)
        psS_cm = tc.tile_pool(name="psS", bufs=1, space="PSUM")
        psS = psS_cm.__enter__()
        pss = psS.tile([1, 512], f32, name="pss")
        for dx in range(7):
            nc.tensor.matmul(pss[:, :], spw_r[:, dx: dx + 1],
                             il[:, :, dx: dx + 32],
                             start=(dx == 0), stop=(dx == 6))
        nc.scalar.activation(sig[:, :], pss[:, :], AF.Sigmoid,
                             scale=bn_sb[0:1, 0:1], bias=bn_sb[0:1, 1:2])
        # broadcast sig along partitions with a rank-1 matmul on the (warm)
        # PE — cheaper than the gpsimd partition_broadcast
        sig_ps = psS.tile([128, 512], f32, name="sig_ps")
        nc.tensor.matmul(sig_ps[:, :], ones_row[:, :], sig[:, :],
                         start=True, stop=True)
        sigv = sig_ps.rearrange("p (r c) -> p r c", c=32)

        if stage == "dbg":
            nc.sync.dma_start(prm["dbg_sig"][:], sig[:])

        # fused = xb_own + xc_own * sigb (all on DVE: gpsimd's 0.42x ALU
        # efficiency would put ~2.2us extra on this critical chain)
        fused = apool.tile([128, 2, 16, 32], f16)
        for j in range(2):
            tm = wk.tile([128, 16, 32], f32, tag="tm", name="tm")
            nc.vector.tensor_mul(tm[:, :, :], xcv[j][:, 3:19, 1:33], sigv)
            nc.vector.tensor_add(fused[:, j, :, :], xbv[j][:, 3:19, 1:33],
                                 tm[:, :, :])
        psS_cm.__exit__(None, None, None)

        if stage == "dbg":
            nc.sync.dma_start(prm["dbg_fused"][:], fused[:])

        # ---- PSP pools (raw block sums over own rows) ----
        pools = wk.tile([128, 2, 43], f32, name="pools")
        for j in range(2):
            f8 = fused[:, j].rearrange("p (rb ri) (cb ci) -> p rb cb ri ci",
                                       ri=4, ci=4)
            p8v = pools[:, j, 11:43].rearrange("p (rb cb) -> p rb cb", cb=8)
            nc.vector.tensor_reduce(p8v, f8, AX.XY, ALU.add)
            p8i = pools[:, j, 11:43].rearrange(
                "p (rb ri cb ci) -> p rb cb ri ci", rb=2, ri=2, cb=4, ci=2)
            p4v = pools[:, j, 3:11].rearrange("p (rb cb) -> p rb cb", cb=4)
            nc.vector.tensor_reduce(p4v, p8i, AX.XY, ALU.add)
        p4i = pools[:, :, 3:11].rearrange(
            "p j (rb cb ci) -> p j cb rb ci", rb=2, cb=2, ci=2)
        nc.vector.tensor_reduce(
            pools[:, :, 1:3].rearrange("p j (a k) -> p j a k", a=2, k=1),
            p4i, AX.XY, ALU.add)
        nc.vector.tensor_reduce(pools[:, :, 0:1], pools[:, :, 1:3], AX.X,
                                ALU.add)
        pools_bf = wk.tile([128, 2, 43], bf, name="pools_bf")
        nc.vector.tensor_copy(pools_bf[:, :, :], pools[:, :, :])

        pools_d = dram.tile([2, 128, 43], bf)
        pools_o = dram.tile([2, 2, 128, 43], bf)
        nc.sync.dma_start(pools_d.rearrange("j p k -> p j k"),
                          pools_bf[:, :, :])
        nc.gpsimd.collective_compute(
            "AllGather", ALU.bypass, replica_groups=PAIRS,
            ins=[pools_d[:].opt()], outs=[pools_o[:].opt()])

        # bottleneck: fused-input chunks accumulate during the AllGather
        out_sb = wk.tile([128, 2, 512], f16, name="out_sb")
        fbv = fused.rearrange("p j r c -> p j (r c)")
        psO = es.enter_context(tc.tile_pool(name="psO", bufs=1, space="PSUM"))
        po = [psO.tile([128, 512], f32, name=f"po{m}") for m in range(2)]
        for m in range(2):
            for k in (2, 3):
                nc.tensor.matmul(po[m][:, :],
                                 bott_wT[:, k, m * 128: m * 128 + 128],
                                 fbv[:, k - 2, :],
                                 start=(k == 2), stop=False)

        # warming across the pools AllGather window: gpsimd rungs + dummy
        # matmuls (the scheduler hoists these into the stats-AllGather window
        # and the pools window, both otherwise idle — net win measured)
        wb = wk.tile([128, 9, 672], bf, name="wb")
        with tc.tile_pool(name="psW3", bufs=1, space="PSUM") as psW3:
            pw3 = psW3.tile([1, 128], f32, name="pw3")
            for r in range(9):
                src = s_b[:, :] if r == 0 else wb[:, r - 1, :]
                nc.gpsimd.tensor_copy(wb[:, r, :], src)
                for i in range(30):
                    nc.tensor.matmul(pw3[0:1, :], wb[:, r, i: i + 1],
                                     wb[:, r, 0:128], start=True, stop=True)

        slp = [wk.tile([128, 2, 43], bf, tag=f"slp{s}", name=f"slp{s}")
               for s in range(2)]
        for s in range(2):
            nc.sync.dma_start(slp[s][:, :, :],
                              pools_o[s].rearrange("j p k -> p j k"))
        # 1x1 convs on pools computed directly in transposed (cell-major)
        # layout: pdT[cell, ch64] = sum_j pools[ch128, cell]^T @ w[ch128,
        # ch64]; slot-1 cells land at partition 32+ via tile_position.
        SI = {1: 0, 2: 1, 4: 2, 8: 3}
        pdA = wk.tile([64, 64], bf, name="pdA")    # s8 cells
        pdC = wk.tile([40, 64], bf, name="pdC")    # s4 cells (gapped)
        pdD = wk.tile([34, 64], bf, name="pdD")    # s2 cells (gapped)
        pdB = wk.tile([1, 64], bf, name="pdB")     # s1 cell
        nc.vector.memset(pdC[:], 0.0)
        nc.vector.memset(pdD[:], 0.0)
        with tc.tile_pool(name="psP", bufs=1, space="PSUM") as psP:
            pdTA = psP.tile([64, 64], f32, name="pdTA")
            pdTC = psP.tile([40, 64], f32, name="pdTC")
            pdTD = psP.tile([34, 64], f32, name="pdTD")
            pdTB = psP.tile([1, 64], f32, name="pdTB")
            for sl in range(2):
                for j in range(2):
                    nc.tensor.matmul(
                        pdTA[32 * sl: 32 * sl + 32, :],
                        slp[sl][:, j, 11:43], psp_wT[:, j, SI[8], :],
                        start=(j == 0), stop=(j == 1),
                        tile_position=(0, 32 * sl))
                for j in range(2):
                    nc.tensor.matmul(
                        pdTC[32 * sl: 32 * sl + 8, :],
                        slp[sl][:, j, 3:11], psp_wT[:, j, SI[4], :],
                        start=(j == 0), stop=(j == 1),
                        tile_position=(0, 32 * sl))
                for j in range(2):
                    nc.tensor.matmul(
                        pdTD[32 * sl: 32 * sl + 2, :],
                        slp[sl][:, j, 1:3], psp_wT[:, j, SI[2], :],
                        start=(j == 0), stop=(j == 1),
                        tile_position=(0, 32 * sl))
            k = 0
            for sl in range(2):
                for j in range(2):
                    nc.tensor.matmul(pdTB[0:1, :], slp[sl][:, j, 0:1],
                                     psp_wT[:, j, SI[1], :],
                                     start=(k == 0), stop=(k == 3))
                    k += 1
            nc.vector.tensor_copy(pdA[:, :], pdTA[:, :])
            nc.scalar.copy(pdC[0:8, :], pdTC[0:8, :])
            nc.scalar.copy(pdC[32:40, :], pdTC[32:40, :])
            nc.vector.tensor_copy(pdD[0:2, :], pdTD[0:2, :])
            nc.vector.tensor_copy(pdD[32:34, :], pdTD[32:34, :])
            nc.scalar.copy(pdB[:, :], pdTB[:, :])

        ones_f = wk.tile([128, 512], bf, name="ones_f")
        nc.vector.memset(ones_f[:], 1.0)
        pri = [wk.tile([128, 512], f16, tag=f"pri{i}", name=f"pri{i}")
               for i in range(2)]
        with tc.tile_pool(name="psR", bufs=2, space="PSUM") as psR:
            pp0 = psR.tile([128, 512], f32, tag="pp", name="pp0")
            nc.tensor.matmul(pp0[0:64, :], pdB[0:1, :], ones_f[0:1, :],
                             start=True, stop=True)
            nc.tensor.matmul(pp0[64:128, :], pdD[0:34, :], Wup[0:34, 0, :],
                             start=True, stop=True, tile_position=(0, 64))
            nc.scalar.copy(pri[0][:, :], pp0[:, :])
            pp1 = psR.tile([128, 512], f32, tag="pp", name="pp1")
            nc.tensor.matmul(pp1[0:64, :], pdC[0:40, :], Wup[0:40, 1, :],
                             start=True, stop=True)
            nc.tensor.matmul(pp1[64:128, :], pdA[0:64, :], Wup[0:64, 2, :],
                             start=True, stop=True, tile_position=(0, 64))
            nc.vector.tensor_copy(pri[1][:, :], pp1[:, :])

        if stage == "dbg":
            nc.sync.dma_start(prm["dbg_pri0"][:], pri[0][:])
            nc.sync.dma_start(prm["dbg_pri1"][:], pri[1][:])

        ov = out_sb.rearrange("p j (r c) -> p j r c", c=32)
        for m in range(2):
            for k in range(2):
                nc.tensor.matmul(po[m][:, :],
                                 bott_wT[:, k, m * 128: m * 128 + 128],
                                 pri[k][:, :],
                                 start=False, stop=(k == 1))
            nc.scalar.activation(out_sb[:, m, :], po[m][:, :], AF.Relu,
                                 bias=bott_b[:, m: m + 1])
            nc.sync.dma_start(out_prm[:, m], ov[:, m])


# ---------------------------------------------------------------------------
# Runner
# ---------------------------------------------------------------------------

_CACHE = {}


def _get_nc(stage="full"):
    if stage not in _CACHE:
        _CACHE[stage] = build(stage)
    return _CACHE[stage]


def run_cores(inputs, stage="full"):
    nc = _get_nc(stage)
    in_maps = [prep_core_inputs(inputs, c) for c in range(N_CORES)]
    res = run_bass_kernel_spmd(nc, in_maps, list(range(N_CORES)))
    return res.results


def kernel(**inputs):
    results = run_cores(inputs, "full")
    out = np.zeros((B, 1, COUT, H, W), np.float32)
    for c in range(N_CORES):
        b, h = c // 2, c % 2
        o = results[c]["out"]                    # [128, 2, 16, 32]
        out[b, 0, :, 16 * h: 16 * h + 16, :] = (
            o.transpose(1, 0, 2, 3).reshape(COUT, 16, 32))
    return out



# revision 95
# speedup vs baseline: 1.0551x; 1.0009x over previous
"""Trainium2 Bass kernel for nn_BottleneckFusion (STCN memory readout + ResBlock
+ CBAM + PSP + bottleneck), 8-core SPMD.

Sharding: core c -> (batch b = c//2, half h = c%2).
  Phase A (attention): TM split across the pair (4 memory frames each); the
    affinity/exp/value loop is software-pipelined (affinity matmuls for slice
    i+1 issue before slice i's value matmuls) and the sumexp accumulates on
    DVE+GpSimd; the flash combine of (unnormalized value, sumexp) is a
    pairwise ReduceScatter that delivers each core exactly its clamped 21-row
    x-window, pre-summed (payload 345KB vs 1MB for the old AllGather).
  Phase B (convs/CBAM/PSP): row-half split with halo recompute. The
    query-side halves of the rb1/rbd convs run under the ReduceScatter in 6
    held PSUM banks; the xb own-row block finishes first so the channel-gate
    stats AllGather (per-channel sum via the activation's accum_out) flies
    while the halo rows convolve; the comp path works on gated xc directly
    (per-partition scale, PE transposes, free-dim maxes, scaled-ones-matmul
    mean); sig broadcasts via gpsimd partition_broadcast; the PSP pool
    AllGather (bf16) overlaps the fused-input chunks of the bottleneck conv,
    and the 1x1 pool convs produce pd directly in transposed (cell-major)
    layout so the upsample needs no PE transposes.

kernel(**inputs) takes the FULL unsharded inputs and returns the FULL output.
"""
import sys

sys.path.insert(0, "/opt/trn_rl_repo")

import numpy as np
import ml_dtypes

import concourse.bass as bass
import concourse.bacc as bacc
import concourse.mybir as mybir
import concourse.tile as tile
from concourse.bass_utils import run_bass_kernel_spmd

BF16 = ml_dtypes.bfloat16
F16 = np.float16
bf = mybir.dt.bfloat16
f16 = mybir.dt.float16
f32 = mybir.dt.float32
AF = mybir.ActivationFunctionType
ALU = mybir.AluOpType
AX = mybir.AxisListType

N_CORES = 8
B, TM, CIN, CK, CV, COUT, H, W = 4, 8, 256, 64, 256, 256, 32, 32
EPS = 1e-5

# local row coordinates: l = image_row - (r0 - 5), l in 0..25
XROWS = 26                 # x window rows (image r0-5 .. r0+20)
CROWS = 22                 # xb/xc/comp local rows (image r0-3 .. r0+18)
PIXPAD = 768               # padded xc free size (22*34=748 -> 768)
PAIRS = [[0, 1], [2, 3], [4, 5], [6, 7]]
UPS = (2, 4, 8)            # upsampled PSP scales
# full pool pyramid offsets [s1, s2, s4, s8] and own-partial offsets
FOFF = {1: 0, 2: 1, 4: 5, 8: 21}
POFF = {1: 0, 2: 1, 4: 3, 8: 11}


def interp_matrix(s_in, s_out=32):
    if s_in == 1:
        return np.ones((s_out, 1), np.float32)
    c = np.arange(s_out) * (s_in - 1) / (s_out - 1)
    lo = np.floor(c).astype(np.int64)
    hi = np.minimum(lo + 1, s_in - 1)
    w = (c - lo).astype(np.float32)
    M = np.zeros((s_out, s_in), np.float32)
    M[np.arange(s_out), lo] += 1.0 - w
    M[np.arange(s_out), hi] += w
    return M


# ---------------------------------------------------------------------------
# Host-side input preparation
# ---------------------------------------------------------------------------

def _pad_hw(a):
    out = np.zeros(a.shape[:-2] + (34, 34), a.dtype)
    out[..., 1:33, 1:33] = a
    return out


def _chw_chunks(a):
    """[256, ...] -> [128, 2, ...] (partition, chunk)."""
    return a.reshape(2, 128, *a.shape[1:]).transpose(
        1, 0, *range(2, a.ndim + 1))


def prep_core_inputs(inputs, core):
    b, h = core // 2, core % 2
    r0 = 16 * h
    g = {}

    f16_q = np.asarray(inputs["f16_q"], np.float32)
    f16_m = np.asarray(inputs["f16_m"], np.float32)
    value_m = np.asarray(inputs["value_m"], np.float32)

    # xm: [128, 2, 4, 34, 34] padded memory frames
    src = f16_m[b, 4 * h: 4 * h + 4]                        # [4, 256, 32, 32]
    src = src.reshape(4, 2, 128, 32, 32).transpose(2, 1, 0, 3, 4)
    g["xm"] = _pad_hw(src).astype(F16)

    # xq: [128, 2, 34, 34] padded query
    q = _chw_chunks(f16_q[b, 0])                            # [128, 2, 32, 32]
    g["xq"] = _pad_hw(q).astype(F16)

    # vT: [128, 32, 256] transposed value
    V = value_m[b][:, 4 * h: 4 * h + 4].reshape(CV, 4096)
    g["vT"] = np.ascontiguousarray(
        V.T.reshape(32, 128, CV).transpose(1, 0, 2)).astype(BF16)  # stays bf16 (matches e)

    # x window q-part: [128, 2, 26, 34]
    qw = np.zeros((128, 2, XROWS, 34), np.float32)
    for l in range(XROWS):
        img = r0 - 5 + l
        if 0 <= img <= 31:
            qw[:, :, l, 1:33] = q[:, :, img, :]
    g["xqb_raw"] = qw.astype(F16)
    g["xqb_relu"] = np.maximum(qw, 0.0).astype(F16)

    pk_w = np.asarray(inputs["pk_w"], np.float32)
    g["pk_wT"] = np.ascontiguousarray(
        pk_w.reshape(CK, 2, 128, 3, 3).transpose(2, 1, 3, 4, 0)).astype(F16)
    pk_b = np.asarray(inputs["pk_b"], np.float32)
    g["pkb2"] = np.concatenate([pk_b, pk_b]).reshape(128, 1).astype(np.float32)

    def conv_lhsT(w, kc):
        co = w.shape[0]
        return np.ascontiguousarray(
            w.reshape(co, kc, 128, 3, 3).transpose(2, 1, 3, 4, 0)).astype(F16)

    g["rb1_wT"] = conv_lhsT(np.asarray(inputs["rb1_w"], np.float32), 4)
    g["rb2_wT"] = conv_lhsT(np.asarray(inputs["rb2_w"], np.float32), 2)
    g["rbd_wT"] = conv_lhsT(np.asarray(inputs["rbd_w"], np.float32), 4)
    g["rb1_b"] = np.asarray(inputs["rb1_b"], np.float32).reshape(2, 128).T.copy()
    g["xb_bias"] = (np.asarray(inputs["rb2_b"], np.float32)
                    + np.asarray(inputs["rbd_b"], np.float32)
                    ).reshape(2, 128).T.copy()

    w1 = np.asarray(inputs["mlp_w1"], np.float32)           # [16, 256]
    g["mlp_w1T"] = np.ascontiguousarray(
        w1.reshape(16, 2, 128).transpose(2, 1, 0)).copy()   # [128, 2, 16]
    g["mlp_w1Ts"] = (g["mlp_w1T"] / 1024.0).copy()          # folds the mean
    g["mlp_b1"] = np.asarray(inputs["mlp_b1"], np.float32).reshape(16, 1).copy()
    g["mlp_w2T"] = np.ascontiguousarray(
        np.asarray(inputs["mlp_w2"], np.float32).T).copy()  # [16, 256]
    g["mlp_b2x2"] = (2.0 * np.asarray(inputs["mlp_b2"], np.float32)
                     ).reshape(2, 128).T.copy()

    spw = np.asarray(inputs["sp_w"], np.float32)[0]       # [2, 7, 7]
    g["spw_r"] = np.ascontiguousarray(
        spw.reshape(14, 7)).astype(np.float16)                # [(ch,dy), dx]
    bn_scale = float(np.asarray(inputs["sp_g"], np.float32)[0]) / float(
        np.sqrt(1.0 + EPS))
    bn_bias = float(np.asarray(inputs["sp_b"], np.float32)[0])
    g["bn_sb"] = np.array([[bn_scale, bn_bias]], np.float32)

    maskT = np.zeros((128, 6, 1), np.float16)
    mask_mean = np.zeros((1, 768), np.float16)
    for pix in range(CROWS * 34):
        img = r0 - 3 + pix // 34
        if 0 <= img <= 31:
            maskT[pix % 128, pix // 128, 0] = 1.0
            mask_mean[0, pix] = 1.0
    g["comp_maskT"] = maskT
    g["mask_mean"] = mask_mean

    pw = np.zeros((128, 2, 4, 64), np.float32)
    for si, s in enumerate((1, 2, 4, 8)):
        wc = np.asarray(inputs[f"psp_w{s}"], np.float32)[:, :, 0, 0]
        scale = 1.0 / ((32 // s) ** 2)
        pw[:, :, si, :] = (wc.T * scale).reshape(2, 128, 64).transpose(1, 0, 2)
    g["psp_wT"] = pw.astype(BF16)

    # folded upsample operators: Wup[row(k), si, (r*32+c)] = M[r0+r, jr] *
    # M[c, jc] where k = jr*s+jc. Rows are gap-laid-out to match the
    # transposed-pd psum layout (slot-1 cells land at partition 32+).
    ROWMAP = {2: lambda k: k if k < 2 else 30 + k,
              4: lambda k: k if k < 8 else 24 + k,
              8: lambda k: k}
    Wup = np.zeros((64, 3, 512), np.float32)
    for si, s in enumerate(UPS):
        M = interp_matrix(s)
        Mrr = M[r0: r0 + 16, :]                 # [16, s]
        for jr in range(s):
            for jc in range(s):
                Wup[ROWMAP[s](jr * s + jc), si, :] = np.outer(
                    Mrr[:, jr], M[:, jc]).reshape(512)
    g["Wup"] = Wup.astype(BF16)

    bott_w = np.asarray(inputs["bott_w"], np.float32)[:, :, 0, 0]
    g["bott_wT"] = np.ascontiguousarray(
        bott_w.reshape(COUT, 4, 128).transpose(2, 1, 0)).astype(F16)
    g["bott_b"] = np.asarray(inputs["bott_b"], np.float32).reshape(2, 128).T.copy()

    rmask = np.zeros((1, XROWS, 34), np.float16)
    for l in range(XROWS):
        if 0 <= r0 - 5 + l <= 31:
            rmask[0, l, :] = 1.0
    g["rmask"] = rmask

    g["ident"] = np.eye(128, dtype=np.float32)
    g["ident_h"] = np.eye(128, dtype=np.float16)
    return g


INPUT_SPECS = [
    ("xm", [128, 2, 4, 34, 34], f16),
    ("xq", [128, 2, 34, 34], f16),
    ("vT", [128, 32, 256], bf),
    ("xqb_raw", [128, 2, XROWS, 34], f16),
    ("xqb_relu", [128, 2, XROWS, 34], f16),
    ("pk_wT", [128, 2, 3, 3, 64], f16),
    ("pkb2", [128, 1], f32),
    ("rb1_wT", [128, 4, 3, 3, 256], f16),
    ("rb2_wT", [128, 2, 3, 3, 256], f16),
    ("rbd_wT", [128, 4, 3, 3, 256], f16),
    ("rb1_b", [128, 2], f32),
    ("xb_bias", [128, 2], f32),
    ("mlp_w1T", [128, 2, 16], f32),
    ("mlp_w1Ts", [128, 2, 16], f32),
    ("mlp_b1", [16, 1], f32),
    ("mlp_w2T", [16, 256], f32),
    ("mlp_b2x2", [128, 2], f32),
    ("spw_r", [14, 7], f16),
    ("bn_sb", [1, 2], f32),
    ("comp_maskT", [128, 6, 1], f16),
    ("mask_mean", [1, 768], f16),
    ("psp_wT", [128, 2, 4, 64], bf),
    ("Wup", [64, 3, 512], bf),
    ("bott_wT", [128, 4, 256], f16),
    ("bott_b", [128, 2], f32),
    ("ident", [128, 128], f32),
    ("ident_h", [128, 128], f16),
    ("rmask", [1, XROWS, 34], f16),
]


# ---------------------------------------------------------------------------
# Device kernel
# ---------------------------------------------------------------------------

def build(stage="full"):
    nc = bacc.Bacc("TRN2", target_bir_lowering=False, debug=False,
                   num_devices=N_CORES)
    prm = {n: nc.declare_dram_parameter(n, sh, dt, isOutput=False)
           for n, sh, dt in INPUT_SPECS}
    if stage == "A":
        out_prm = nc.declare_dram_parameter("out_a", [257, 1024], f32,
                                            isOutput=True)
    else:
        out_prm = nc.declare_dram_parameter("out", [128, 2, 16, 32], f16,
                                            isOutput=True)
    if stage == "dbg":
        for n, sh, dt in [("dbg_xraw", [128, 4, XROWS, 34], f16),
                          ("dbg_xb", [128, 2, CROWS, 34], f32),
                          ("dbg_gate", [128, 2, 1], f32),
                          ("dbg_sig", [1, 512], bf),
                          ("dbg_fused", [128, 2, 16, 32], f16),
                          ("dbg_pd", [64, 85], f32),
                          ("dbg_pri0", [128, 512], f16),
                          ("dbg_pri1", [128, 512], f16)]:
            prm[n] = nc.declare_dram_parameter(n, sh, dt, isOutput=True)
    with tile.TileContext(nc) as tc:
        _emit(tc, nc, prm, stage, out_prm)
    nc.compile()
    return nc


def _emit(tc, nc, prm, stage, out_prm):
    import contextlib
    es = contextlib.ExitStack()
    with es:
        wpool = es.enter_context(tc.tile_pool(name="wpool", bufs=1))
        apool = es.enter_context(tc.tile_pool(name="apool", bufs=1))
        dram = es.enter_context(tc.tile_pool(name="dram", bufs=1, space="DRAM"))
        aonly_cm = tc.tile_pool(name="aonly", bufs=1)
        aonly = aonly_cm.__enter__()

        def load(name, pool=wpool):
            t = pool.tile(list(prm[name].shape), prm[name].dtype,
                          name=f"{name}_sb")
            nc.sync.dma_start(t[:], prm[name][:])
            return t

        pk_wT = wpool.tile([128, 2, 3, 3, 64], f16, name="pk_wT_sb")
        for j in range(2):
            for dy in range(3):
                nc.sync.dma_start(pk_wT[:, j, dy], prm["pk_wT"][:, j, dy])
        pkb2 = load("pkb2")
        xq_sb = aonly.tile([128, 2, 34, 34], f16, name="xq_sb")
        for j in range(2):
            nc.gpsimd.dma_start(xq_sb[:, j], prm["xq"][:, j])
        xm_sb = aonly.tile([128, 2, 4, 34, 34], f16, name="xm_sb")
        for t in range(4):
            nc.sync.dma_start(xm_sb[:, :, t, :, :], prm["xm"][:, :, t, :, :])
        vT_sb = load("vT", aonly)

        # phase-B weights: load early (DMA bandwidth is free during phase A,
        # and apool space is disjoint from the phase-A-only arena)
        rb1_wT = load("rb1_wT", apool)
        rb2_wT = load("rb2_wT", apool)
        rbd_wT = load("rbd_wT", apool)
        rb1_b = load("rb1_b", apool)
        xb_bias = load("xb_bias", apool)
        mlp_w1T = load("mlp_w1T", apool)
        mlp_w1Ts = load("mlp_w1Ts", apool)
        mlp_b1 = load("mlp_b1", apool)
        mlp_w2T = load("mlp_w2T", apool)
        mlp_b2x2 = load("mlp_b2x2", apool)
        spw_r = load("spw_r", apool)
        bn_sb = load("bn_sb", apool)
        comp_maskT = load("comp_maskT", apool)
        mask_mean = load("mask_mean", apool)
        psp_wT = load("psp_wT", apool)
        Wup = load("Wup", apool)
        bott_wT = load("bott_wT", apool)
        bott_b = load("bott_b", apool)
        ident = load("ident", apool)
        ident_h = load("ident_h", apool)

        ones_bf = wpool.tile([128, 1], bf)
        nc.vector.memset(ones_bf[:], 1.0)

        # im2col buffer for the 7x7 spatial conv: zero the pad columns early
        il = wpool.tile([14, 16, 38], f16, name="il")
        nc.vector.memset(il[:, :, 0:2], 0.0)
        nc.vector.memset(il[:, :, 36:38], 0.0)

        # ================= phase A =================
        mk_sb = aonly.tile([128, 2, 1024], f16)
        qk_sb = aonly.tile([128, 1024], f16)

        # psAff/psV opened BEFORE psA so their banks are disjoint from the
        # conv psum banks — the affinity/value matmuls then start without a
        # bank-reuse WAR stall on the last mk activation
        psAff_cm = tc.tile_pool(name="psAff", bufs=2, space="PSUM")
        psAff = psAff_cm.__enter__()
        psV_cm = tc.tile_pool(name="psV", bufs=1, space="PSUM")
        psV = psV_cm.__enter__()
        with tc.tile_pool(name="psA", bufs=2, space="PSUM") as psA:
            for n in range(2):
                pq = psA.tile([128, 512], f32, tag="mkps", name="pq")
                k = 0
                for j in range(2):
                    for dy in range(3):
                        for dx in range(3):
                            nc.tensor.matmul(
                                pq[0:64, :], pk_wT[:, j, dy, dx, :],
                                xq_sb[:, j, n * 16 + dy: n * 16 + dy + 16,
                                      dx: dx + 32],
                                start=(k == 0), stop=(k == 17))
                            k += 1
                nc.scalar.activation(
                    qk_sb[0:64, n * 512: (n + 1) * 512], pq[0:64, :],
                    AF.Identity, bias=pkb2[0:64, 0:1])
            # replicate qk to partitions 64..127 so odd-frame mk slices
            # (base partition 64) can stream against it
            nc.sync.dma_start(qk_sb[64:128, :], qk_sb[0:64, :])

            for tp in range(2):
                for n in range(2):
                    pm = psA.tile([128, 512], f32, tag="mkps", name="pm")
                    for par in range(2):
                        t = 2 * tp + par
                        k = 0
                        for j in range(2):
                            for dy in range(3):
                                for dx in range(3):
                                    nc.tensor.matmul(
                                        pm[64 * par: 64 * par + 64, :],
                                        pk_wT[:, j, dy, dx, :],
                                        xm_sb[:, j, t,
                                              n * 16 + dy: n * 16 + dy + 16,
                                              dx: dx + 32],
                                        start=(k == 0), stop=(k == 17),
                                        tile_position=(0, 64 * par),
                                    )
                                    k += 1
                    nc.scalar.activation(
                        mk_sb[:, tp, n * 512: (n + 1) * 512], pm[:, :],
                        AF.Identity, bias=pkb2[:, 0:1])

        # flash-combine via pairwise ReduceScatter: each core receives the
        # summed (value, sumexp) restricted to ITS x-window clamped to the
        # image: half hh covers img rows max(0,16hh-5)..min(31,16hh+20),
        # i.e. 21 rows = px [352*hh : 352*hh + 672].
        arv_in = dram.tile([2, 257, 672], bf)
        arv_out = dram.tile([257, 672], bf)

        if True:
            vps = [psV.tile([128, 1024], f32, name=f"vps{j}") for j in range(2)]
            s_acc = aonly.tile([128, 1024], f32, name="s_acc")
            s_accb = aonly.tile([128, 1024], f32, name="s_accb")

            order = [16 * h + o + 8 * par for h in range(2) for o in range(8)
                     for par in range(2)]

            # software-pipelined: issue slice idx+1's affinity matmuls before
            # slice idx's value matmuls so the PE never waits on the Exp
            def aff_exp(idx):
                i = order[idx]
                t = i >> 3
                pb = i & 7
                tp, par = t >> 1, t & 1
                lhs_aff = mk_sb[64 * par: 64 * par + 64, tp,
                                pb * 128: pb * 128 + 128]
                e_t = aonly.tile([128, 1024], bf, tag="e", name="e_t", bufs=4)
                for qn in range(2):
                    pa = psAff.tile([128, 512], f32, tag="affp", name="pa")
                    nc.tensor.matmul(
                        pa[:, :], lhs_aff,
                        qk_sb[64 * par: 64 * par + 64,
                              qn * 512: (qn + 1) * 512],
                        start=True, stop=True)
                    nc.scalar.activation(
                        e_t[:, qn * 512: (qn + 1) * 512], pa[:, :],
                        AF.Exp, scale=0.125)
                return e_t

            e_cur = aff_exp(0)
            for idx in range(32):
                i = order[idx]
                e_t = e_cur
                if idx < 31:
                    e_cur = aff_exp(idx + 1)
                for j in range(2):
                    for qn in range(2):
                        nc.tensor.matmul(
                            vps[j][:, qn * 512: (qn + 1) * 512],
                            vT_sb[:, i, j * 128: (j + 1) * 128],
                            e_t[:, qn * 512: (qn + 1) * 512],
                            start=(idx == 0), stop=(idx == 31))
                # split the sumexp accumulation DVE/Pool (Pool runs Add at
                # 0.42 efficiency, so give it the smaller share)
                on_pool = idx % 3 == 2
                acc = s_accb if on_pool else s_acc
                eng = nc.gpsimd if on_pool else nc.vector
                if idx == (2 if on_pool else 0):
                    eng.tensor_copy(acc[:, :], e_t[:, :])
                else:
                    eng.tensor_add(acc[:, :], acc[:, :], e_t[:, :])

            v_sb = aonly.tile([128, 2, 1024], bf, name="v_sb")
            s_sb = aonly.tile([1, 1024], bf, name="s_sb")
            nc.vector.tensor_copy(v_sb[:, 0, :], vps[0][:, :])
            nc.scalar.copy(v_sb[:, 1, :], vps[1][:, :])
            # fold the 128-partition sumexp accumulators off the PE:
            # elementwise add on DVE, then a gpsimd partition reduction
            import concourse.bass_isa as bass_isa
            nc.vector.tensor_add(s_acc[:, :], s_acc[:, :], s_accb[:, :])
            s_red = aonly.tile([128, 1024], f32, name="s_red")
            nc.gpsimd.partition_all_reduce(s_red[:, :], s_acc[:, :], 128,
                                           bass_isa.ReduceOp.add)
            nc.vector.tensor_copy(s_sb[:, :], s_red[0:1, :])
            for hh in range(2):
                eng = nc.sync if hh == 0 else nc.scalar
                for j in range(2):
                    eng.dma_start(
                        arv_in[hh, 128 * j: 128 * j + 128, :],
                        v_sb[:, j, 352 * hh: 352 * hh + 672])
                eng.dma_start(arv_in[hh, 256:257, :],
                              s_sb[:, 352 * hh: 352 * hh + 672])

        psV_cm.__exit__(None, None, None)
        psAff_cm.__exit__(None, None, None)

        nc.gpsimd.collective_compute(
            "ReduceScatter", ALU.add, replica_groups=PAIRS,
            ins=[arv_in[:].opt()], outs=[arv_out[:].opt()])

        aonly_cm.__exit__(None, None, None)

        # ================= phase B =================
        wk = es.enter_context(tc.tile_pool(name="wk", bufs=1))

        x_raw = apool.tile([128, 4, XROWS, 34], f16)
        x_relu = apool.tile([128, 4, XROWS, 34], f16)
        for tt in (x_raw, x_relu):
            nc.vector.memset(tt[:, 2:4, :, :], 0.0)
        nc.sync.dma_start(x_raw[:, 0:2, :, :], prm["xqb_raw"][:])
        nc.sync.dma_start(x_relu[:, 0:2, :, :], prm["xqb_relu"][:])

        r1_relu = apool.tile([128, 2, XROWS, 34], f16)
        nc.vector.memset(r1_relu[:, :, 0:1, :], 0.0)
        nc.vector.memset(r1_relu[:, :, 25:26, :], 0.0)
        nc.vector.memset(r1_relu[:, :, :, 0:1], 0.0)
        nc.vector.memset(r1_relu[:, :, :, 33:34], 0.0)
        rmaskb = apool.tile([128, XROWS, 34], f16)
        nc.sync.dma_start(rmaskb[:], prm["rmask"][:].partition_broadcast(128))
        xb = apool.tile([128, 2, PIXPAD], f32)
        xbv = [xb[:, j, 0: CROWS * 34].rearrange("p (r c) -> p r c", c=34)
               for j in range(2)]
        for j in range(2):
            nc.vector.memset(xbv[j][:, :, 0:1], 0.0)
            nc.vector.memset(xbv[j][:, :, 33:34], 0.0)
        nc.vector.memset(xb[:, :, CROWS * 34:], 0.0)

        # conv row blocks: r1 full; xb own rows (xbv 3..18) first so the
        # stats AllGather can launch while the halo rows (0..2, 19..21) are
        # still convolving.
        R1BLK = ((1, 16), (17, 8))
        stats = wk.tile([128, 2, 2], f32, name="stats")
        stats_d = dram.tile([256, 2], f32)
        stats_o = dram.tile([2, 256, 2], f32)

        def conv9(ps, wT, src, m, j, l0, nr, k0, stop_k):
            k = k0
            for dy in range(3):
                for dx in range(3):
                    nc.tensor.matmul(
                        ps[:, : nr * 32],
                        wT[:, j, dy, dx, m * 128: m * 128 + 128],
                        src[:, j, l0 + dy - 1: l0 + dy - 1 + nr, dx: dx + 32],
                        start=(k == 0), stop=(k == stop_k))
                    k += 1
            return k

        psX_cm = tc.tile_pool(name="psX", bufs=1, space="PSUM")
        psX = psX_cm.__enter__()
        psR1_cm = tc.tile_pool(name="psR1", bufs=1, space="PSUM")
        psR1 = psR1_cm.__enter__()
        pr = [[psR1.tile([128, nr * 32], f32, name=f"pr{m}{bi}")
               for bi, (l0, nr) in enumerate(R1BLK)] for m in range(2)]
        pxo = [psX.tile([128, 512], f32, name=f"pxo{m}") for m in range(2)]

        # -- query-side partial convs: run while the ReduceScatter flies
        for m in range(2):
            for bi, (l0, nr) in enumerate(R1BLK):
                k = 0
                for j in range(2):
                    k = conv9(pr[m][bi], rb1_wT, x_relu, m, j, l0, nr, k, -1)
        for m in range(2):
            k = 0
            for j in range(2):
                k = conv9(pxo[m], rbd_wT, x_raw, m, j, 5, 16, k, -1)

        # -- val window lands (RS output), normalize, build x val chunks at
        # the per-core row offset (h=0: x rows 5..25, h=1: x rows 0..20)
        val_win = wk.tile([128, 2, 672], bf, name="val_win")
        s_b = wk.tile([128, 672], bf, name="s_b")
        nc.scalar.dma_start(s_b[:],
                            arv_out[256:257, :].partition_broadcast(128))
        nc.sync.dma_start(val_win[:, 0, :], arv_out[0:128, :])
        nc.scalar.dma_start(val_win[:, 1, :], arv_out[128:256, :])
        # warm the PE (p-state) while the val window is normalized
        with tc.tile_pool(name="psW2", bufs=1, space="PSUM") as psW2:
            pw2 = psW2.tile([1, 128], f32, name="pw2")
            for i in range(48):
                nc.tensor.matmul(pw2[0:1, :], s_b[:, i: i + 1],
                                 s_b[:, 0:128], start=True, stop=True)
        invb = wk.tile([128, 672], f32, name="invb")
        nc.vector.reciprocal(invb[:, :], s_b[:, :])
        voff = (1 - nc.vector.partition_id() % 2) * 5
        voff_a = (1 - nc.scalar.partition_id() % 2) * 5
        for j in range(2):
            vv = val_win[:, j, :].rearrange("p (r c) -> p r c", c=32)
            iv = invb.rearrange("p (r c) -> p r c", c=32)
            nc.vector.tensor_mul(
                x_raw[:, 2 + j, bass.ds(voff, 21), 1:33], vv, iv)
            nc.scalar.activation(
                x_relu[:, 2 + j, bass.ds(voff_a, 21), 1:33],
                x_raw[:, 2 + j, bass.ds(voff_a, 21), 1:33], AF.Relu)

        # -- finish r1 with the val-side chunks
        for m in range(2):
            for bi, (l0, nr) in enumerate(R1BLK):
                k = 18
                for j in (2, 3):
                    k = conv9(pr[m][bi], rb1_wT, x_relu, m, j, l0, nr, k, 35)
                nc.scalar.activation(
                    r1_relu[:, m, l0: l0 + nr, 1:33],
                    pr[m][bi][:, : nr * 32],
                    AF.Relu, bias=rb1_b[:, m: m + 1])
                nc.vector.tensor_mul(r1_relu[:, m, l0: l0 + nr, 1:33],
                                     r1_relu[:, m, l0: l0 + nr, 1:33],
                                     rmaskb[:, l0: l0 + nr, 1:33])
        psR1_cm.__exit__(None, None, None)

        # -- xb own rows: rbd val-side + rb2(r1) -> stats -> AllGather
        # (the per-channel sum falls out of the activation's accum_out)
        for m in range(2):
            k = 18
            for j in (2, 3):
                k = conv9(pxo[m], rbd_wT, x_raw, m, j, 5, 16, k, -1)
            for j in range(2):
                k = conv9(pxo[m], rb2_wT, r1_relu, m, j, 5, 16, k, 53)
            nc.scalar.activation(
                xbv[m][:, 3:19, 1:33], pxo[m][:, :],
                AF.Identity, bias=xb_bias[:, m: m + 1],
                accum_out=stats[:, m, 0:1])
        psX_cm.__exit__(None, None, None)
        for j in range(2):
            nc.vector.tensor_reduce(stats[:, j, 1:2],
                                    xbv[j][:, 3:19, 1:33], AX.XY, ALU.max)
        nc.sync.dma_start(stats_d.rearrange("(j p) k -> p j k", j=2),
                          stats[:, :, :])
        nc.gpsimd.collective_compute(
            "AllGather", ALU.bypass, replica_groups=PAIRS,
            ins=[stats_d[:].opt()], outs=[stats_o[:].opt()])

        # -- halo rows (full conv) run under the stats AllGather, in the
        # banks freed by r1
        with tc.tile_pool(name="psH", bufs=1, space="PSUM") as psH:
            pxh = [[psH.tile([128, 96], f32, name=f"pxh{m}{ci}")
                    for ci in range(2)] for m in range(2)]
            for m in range(2):
                for ci, l0 in enumerate((2, 21)):
                    k = 0
                    for j in range(4):
                        src = x_raw
                        k = conv9(pxh[m][ci], rbd_wT, src, m, j, l0, 3, k, -1)
                    for j in range(2):
                        k = conv9(pxh[m][ci], rb2_wT, r1_relu, m, j, l0, 3,
                                  k, 53)
                    nc.scalar.activation(
                        xbv[m][:, l0 - 2: l0 + 1, 1:33],
                        pxh[m][ci][:, :],
                        AF.Identity, bias=xb_bias[:, m: m + 1])

        wa = wk.tile([128, 3, 1768], f16, name="wa")

        if stage == "dbg":
            nc.sync.dma_start(prm["dbg_xraw"][:], x_raw[:])
            for j in range(2):
                nc.sync.dma_start(prm["dbg_xb"][:, j], xbv[j])

        # ---- CBAM channel gate (stats AllGather result) ----
        sl = wk.tile([128, 2, 2, 2], f32, name="sl")  # [p, slot, j, (sum,max)]
        nc.sync.dma_start(sl[:, :, :, :],
                          stats_o.rearrange("s (j p) k -> p s j k", j=2))
        # mean path rides the matmul accumulation (W1/1024 pre-scaled on the
        # host); only the max path needs a combine op
        gmax = wk.tile([128, 2, 1], f32, name="gmax")
        nc.vector.tensor_max(gmax[:, :, :], sl[:, 0, :, 1:2], sl[:, 1, :, 1:2])

        gate = wk.tile([128, 2, 1], f32, name="gate")
        with tc.tile_pool(name="psG", bufs=1, space="PSUM") as psG:
            ph1 = psG.tile([16, 2], f32, name="ph1")
            k = 0
            for s in range(2):
                for j in range(2):
                    nc.tensor.matmul(ph1[:, 0:1], mlp_w1Ts[:, j, :],
                                     sl[:, s, j, 0:1],
                                     start=(k == 0), stop=(k == 3))
                    k += 1
            for j in range(2):
                nc.tensor.matmul(ph1[:, 1:2], mlp_w1T[:, j, :],
                                 gmax[:, j, :], start=(j == 0), stop=(j == 1))
            h1 = wk.tile([16, 2], f32, name="h1")
            nc.scalar.activation(h1[:, :], ph1[:, :], AF.Relu,
                                 bias=mlp_b1[:, 0:1])
            # W2.relu(h_mean) + W2.relu(h_max) = W2.(relu(h_mean)+relu(h_max))
            hs = wk.tile([16, 1], f32, name="hs")
            nc.vector.tensor_add(hs[:, :], h1[:, 0:1], h1[:, 1:2])
            for j in range(2):
                ph2 = psG.tile([128, 1], f32, tag="ph2", name="ph2")
                nc.tensor.matmul(ph2[:, :], mlp_w2T[:, j * 128: j * 128 + 128],
                                 hs[:, :], start=True, stop=True)
                nc.scalar.activation(gate[:, j, :], ph2[:, :], AF.Sigmoid,
                                     bias=mlp_b2x2[:, j: j + 1])

        if stage == "dbg":
            nc.sync.dma_start(prm["dbg_gate"][:], gate[:])

        # ---- xc = gate * xb (per-partition scale), then pixel-major copies
        xc = apool.tile([128, 2, 768], f16)
        nc.vector.memset(xc[:, :, 748:768], 0.0)
        for (c0, c1) in ((0, 384), (384, 748)):
            nc.scalar.mul(xc[:, 0, c0: c1], xb[:, 0, c0: c1],
                          gate[:, 0, 0:1])
            nc.vector.tensor_scalar(xc[:, 1, c0: c1], xb[:, 1, c0: c1],
                                    gate[:, 1, 0:1], None, ALU.mult)
        xcv = [xc[:, j, 0: CROWS * 34].rearrange("p (r c) -> p r c", c=34)
               for j in range(2)]
        # channel max of xc via gpsimd partition reductions (row-major result
        # lands directly on partition 0 — no PE transposes needed)
        cmx = wk.tile([128, 2, 748], f16, name="cmx")
        for j in range(2):
            nc.gpsimd.partition_all_reduce(cmx[:, j, :], xc[:, j, 0:748],
                                           128, bass_isa.ReduceOp.max)
        comp_max = wk.tile([1, 748], f16, name="comp_max")
        nc.vector.tensor_max(comp_max[:, :], cmx[0:1, 0, :], cmx[0:1, 1, :])
        nc.vector.tensor_mul(comp_max[:, :], comp_max[:, :],
                             mask_mean[:, 0:748])
        mean_sb = wk.tile([1, 748], f16, name="mean_sb")
        onesd = wk.tile([128, 1], f16, name="onesd")
        nc.vector.memset(onesd[:], 1.0 / 256.0)
        with tc.tile_pool(name="psM", bufs=1, space="PSUM") as psM:
            pm1 = psM.tile([1, 748], f32, name="pm1")
            for j in range(2):
                for (o0, nn) in ((0, 512), (512, 236)):
                    nc.tensor.matmul(pm1[0:1, o0: o0 + nn],
                                     onesd[:, 0:1],
                                     xc[:, j, o0: o0 + nn],
                                     start=(j == 0), stop=(j == 1))
            nc.scalar.copy(mean_sb[:, :], pm1[:, :])
        nc.vector.tensor_mul(mean_sb[:, :], mean_sb[:, :],
                             mask_mean[:, 0:748])

        comp_flat = dram.tile([2, 768], f16)
        nc.sync.dma_start(comp_flat[0, 0:748], comp_max[:, :])
        nc.scalar.dma_start(comp_flat[1, 0:748], mean_sb[:, :])
        # gather the 7x7-conv im2col rows straight from comp_flat; the L/R
        # zero-pad columns of il were pre-zeroed at kernel start
        for ch in range(2):
            eng = nc.sync if ch == 0 else nc.scalar
            eng.dma_start(
                il[7 * ch: 7 * ch + 7, :, 2:36],
                bass.AP(comp_flat.tensor, 768 * ch,
                        [[34, 7], [34, 16], [1, 34]]))

        # keep the PE out of its low p-state across the ~5us im2col DMA wait:
        # an Activation-timed ladder gates short dummy matmuls so the tensor
        # engine stays continuously busy until the spatial conv's data lands
        wsrc = x_raw[:, 0:2].rearrange("p j r c -> p (j r c)")
        with tc.tile_pool(name="psW", bufs=1, space="PSUM") as psW:
            pw = psW.tile([1, 128], f32, name="pw")
            for r in range(3):
                src = wsrc if r == 0 else wa[:, r - 1, :]
                nc.scalar.activation(wa[:, r, :], src, AF.Identity)
                for i in range(24):
                    nc.tensor.matmul(pw[0:1, :], wa[:, r, i: i + 1],
                                     wa[:, r, 0:128], start=True, stop=True)
        sig = wk.tile([1, 512], bf, name="sig")
        ones_row = wk.tile([1, 128], bf, name="ones_row")
        nc.vector.memset(ones_row[:], 1.0)
        psS_cm = tc.tile_pool(name="psS", bufs=1, space="PSUM")
        psS = psS_cm.__enter__()
        pss = psS.tile([1, 512], f32, name="pss")
        for dx in range(7):
            nc.tensor.matmul(pss[:, :], spw_r[:, dx: dx + 1],
                             il[:, :, dx: dx + 32],
                             start=(dx == 0), stop=(dx == 6))
        nc.scalar.activation(sig[:, :], pss[:, :], AF.Sigmoid,
                             scale=bn_sb[0:1, 0:1], bias=bn_sb[0:1, 1:2])
        # broadcast sig along partitions with a rank-1 matmul on the (warm)
        # PE — cheaper than the gpsimd partition_broadcast
        sig_ps = psS.tile([128, 512], f32, name="sig_ps")
        nc.tensor.matmul(sig_ps[:, :], ones_row[:, :], sig[:, :],
                         start=True, stop=True)
        sigv = sig_ps.rearrange("p (r c) -> p r c", c=32)

        if stage == "dbg":
            nc.sync.dma_start(prm["dbg_sig"][:], sig[:])

        # fused = xb_own + xc_own * sigb (all on DVE: gpsimd's 0.42x ALU
        # efficiency would put ~2.2us extra on this critical chain)
        fused = apool.tile([128, 2, 16, 32], f16)
        for j in range(2):
            tm = wk.tile([128, 16, 32], f32, tag="tm", name="tm")
            nc.vector.tensor_mul(tm[:, :, :], xcv[j][:, 3:19, 1:33], sigv)
            nc.vector.tensor_add(fused[:, j, :, :], xbv[j][:, 3:19, 1:33],
                                 tm[:, :, :])
        psS_cm.__exit__(None, None, None)

        if stage == "dbg":
            nc.sync.dma_start(prm["dbg_fused"][:], fused[:])

        # ---- PSP pools (raw block sums over own rows) ----
        pools = wk.tile([128, 2, 43], f32, name="pools")
        for j in range(2):
            f8 = fused[:, j].rearrange("p (rb ri) (cb ci) -> p rb cb ri ci",
                                       ri=4, ci=4)
            p8v = pools[:, j, 11:43].rearrange("p (rb cb) -> p rb cb", cb=8)
            nc.vector.tensor_reduce(p8v, f8, AX.XY, ALU.add)
            p8i = pools[:, j, 11:43].rearrange(
                "p (rb ri cb ci) -> p rb cb ri ci", rb=2, ri=2, cb=4, ci=2)
            p4v = pools[:, j, 3:11].rearrange("p (rb cb) -> p rb cb", cb=4)
            nc.vector.tensor_reduce(p4v, p8i, AX.XY, ALU.add)
        p4i = pools[:, :, 3:11].rearrange(
            "p j (rb cb ci) -> p j cb rb ci", rb=2, cb=2, ci=2)
        nc.vector.tensor_reduce(
            pools[:, :, 1:3].rearrange("p j (a k) -> p j a k", a=2, k=1),
            p4i, AX.XY, ALU.add)
        nc.vector.tensor_reduce(pools[:, :, 0:1], pools[:, :, 1:3], AX.X,
                                ALU.add)
        pools_bf = wk.tile([128, 2, 43], bf, name="pools_bf")
        nc.vector.tensor_copy(pools_bf[:, :, :], pools[:, :, :])

        pools_d = dram.tile([2, 128, 43], bf)
        pools_o = dram.tile([2, 2, 128, 43], bf)
        nc.sync.dma_start(pools_d.rearrange("j p k -> p j k"),
                          pools_bf[:, :, :])
        nc.gpsimd.collective_compute(
            "AllGather", ALU.bypass, replica_groups=PAIRS,
            ins=[pools_d[:].opt()], outs=[pools_o[:].opt()])

        # bottleneck: fused-input chunks accumulate during the AllGather
        out_sb = wk.tile([128, 2, 512], f16, name="out_sb")
        fbv = fused.rearrange("p j r c -> p j (r c)")
        psO = es.enter_context(tc.tile_pool(name="psO", bufs=1, space="PSUM"))
        po = [psO.tile([128, 512], f32, name=f"po{m}") for m in range(2)]
        for m in range(2):
            for k in (2, 3):
                nc.tensor.matmul(po[m][:, :],
                                 bott_wT[:, k, m * 128: m * 128 + 128],
                                 fbv[:, k - 2, :],
                                 start=(k == 2), stop=False)

        # warming across the pools AllGather window: gpsimd rungs + dummy
        # matmuls (the scheduler hoists these into the stats-AllGather window
        # and the pools window, both otherwise idle — net win measured)
        wb = wk.tile([128, 9, 672], bf, name="wb")
        with tc.tile_pool(name="psW3", bufs=1, space="PSUM") as psW3:
            pw3 = psW3.tile([1, 128], f32, name="pw3")
            for r in range(9):
                src = s_b[:, :] if r == 0 else wb[:, r - 1, :]
                nc.gpsimd.tensor_copy(wb[:, r, :], src)
                for i in range(24):
                    nc.tensor.matmul(pw3[0:1, :], wb[:, r, i: i + 1],
                                     wb[:, r, 0:128], start=True, stop=True)

        slp = [wk.tile([128, 2, 43], bf, tag=f"slp{s}", name=f"slp{s}")
               for s in range(2)]
        for s in range(2):
            nc.sync.dma_start(slp[s][:, :, :],
                              pools_o[s].rearrange("j p k -> p j k"))
        # 1x1 convs on pools computed directly in transposed (cell-major)
        # layout: pdT[cell, ch64] = sum_j pools[ch128, cell]^T @ w[ch128,
        # ch64]; slot-1 cells land at partition 32+ via tile_position.
        SI = {1: 0, 2: 1, 4: 2, 8: 3}
        pdA = wk.tile([64, 64], bf, name="pdA")    # s8 cells
        pdC = wk.tile([40, 64], bf, name="pdC")    # s4 cells (gapped)
        pdD = wk.tile([34, 64], bf, name="pdD")    # s2 cells (gapped)
        pdB = wk.tile([1, 64], bf, name="pdB")     # s1 cell
        nc.vector.memset(pdC[:], 0.0)
        nc.vector.memset(pdD[:], 0.0)
        with tc.tile_pool(name="psP", bufs=1, space="PSUM") as psP:
            pdTA = psP.tile([64, 64], f32, name="pdTA")
            pdTC = psP.tile([40, 64], f32, name="pdTC")
            pdTD = psP.tile([34, 64], f32, name="pdTD")
            pdTB = psP.tile([1, 64], f32, name="pdTB")
            for sl in range(2):
                for j in range(2):
                    nc.tensor.matmul(
                        pdTA[32 * sl: 32 * sl + 32, :],
                        slp[sl][:, j, 11:43], psp_wT[:, j, SI[8], :],
                        start=(j == 0), stop=(j == 1),
                        tile_position=(0, 32 * sl))
                for j in range(2):
                    nc.tensor.matmul(
                        pdTC[32 * sl: 32 * sl + 8, :],
                        slp[sl][:, j, 3:11], psp_wT[:, j, SI[4], :],
                        start=(j == 0), stop=(j == 1),
                        tile_position=(0, 32 * sl))
                for j in range(2):
                    nc.tensor.matmul(
                        pdTD[32 * sl: 32 * sl + 2, :],
                        slp[sl][:, j, 1:3], psp_wT[:, j, SI[2], :],
                        start=(j == 0), stop=(j == 1),
                        tile_position=(0, 32 * sl))
            k = 0
            for sl in range(2):
                for j in range(2):
                    nc.tensor.matmul(pdTB[0:1, :], slp[sl][:, j, 0:1],
                                     psp_wT[:, j, SI[1], :],
                                     start=(k == 0), stop=(k == 3))
                    k += 1
            nc.vector.tensor_copy(pdA[:, :], pdTA[:, :])
            nc.scalar.copy(pdC[0:8, :], pdTC[0:8, :])
            nc.scalar.copy(pdC[32:40, :], pdTC[32:40, :])
            nc.vector.tensor_copy(pdD[0:2, :], pdTD[0:2, :])
            nc.vector.tensor_copy(pdD[32:34, :], pdTD[32:34, :])
            nc.scalar.copy(pdB[:, :], pdTB[:, :])

        ones_f = wk.tile([128, 512], bf, name="ones_f")
        nc.vector.memset(ones_f[:], 1.0)
        pri = [wk.tile([128, 512], f16, tag=f"pri{i}", name=f"pri{i}")
               for i in range(2)]
        with tc.tile_pool(name="psR", bufs=2, space="PSUM") as psR:
            pp0 = psR.tile([128, 512], f32, tag="pp", name="pp0")
            nc.tensor.matmul(pp0[0:64, :], pdB[0:1, :], ones_f[0:1, :],
                             start=True, stop=True)
            nc.tensor.matmul(pp0[64:128, :], pdD[0:34, :], Wup[0:34, 0, :],
                             start=True, stop=True, tile_position=(0, 64))
            nc.scalar.copy(pri[0][:, :], pp0[:, :])
            pp1 = psR.tile([128, 512], f32, tag="pp", name="pp1")
            nc.tensor.matmul(pp1[0:64, :], pdC[0:40, :], Wup[0:40, 1, :],
                             start=True, stop=True)
            nc.tensor.matmul(pp1[64:128, :], pdA[0:64, :], Wup[0:64, 2, :],
                             start=True, stop=True, tile_position=(0, 64))
            nc.vector.tensor_copy(pri[1][:, :], pp1[:, :])

        if stage == "dbg":
            nc.sync.dma_start(prm["dbg_pri0"][:], pri[0][:])
            nc.sync.dma_start(prm["dbg_pri1"][:], pri[1][:])

        ov = out_sb.rearrange("p j (r c) -> p j r c", c=32)
        for m in range(2):
            for k in range(2):
                nc.tensor.matmul(po[m][:, :],
                                 bott_wT[:, k, m * 128: m * 128 + 128],
                                 pri[k][:, :],
                                 start=False, stop=(k == 1))
            nc.scalar.activation(out_sb[:, m, :], po[m][:, :], AF.Relu,
                                 bias=bott_b[:, m: m + 1])
            nc.sync.dma_start(out_prm[:, m], ov[:, m])


# ---------------------------------------------------------------------------
# Runner
# ---------------------------------------------------------------------------

_CACHE = {}


def _get_nc(stage="full"):
    if stage not in _CACHE:
        _CACHE[stage] = build(stage)
    return _CACHE[stage]


def run_cores(inputs, stage="full"):
    nc = _get_nc(stage)
    in_maps = [prep_core_inputs(inputs, c) for c in range(N_CORES)]
    res = run_bass_kernel_spmd(nc, in_maps, list(range(N_CORES)))
    return res.results


def kernel(**inputs):
    results = run_cores(inputs, "full")
    out = np.zeros((B, 1, COUT, H, W), np.float32)
    for c in range(N_CORES):
        b, h = c // 2, c % 2
        o = results[c]["out"]                    # [128, 2, 16, 32]
        out[b, 0, :, 16 * h: 16 * h + 16, :] = (
            o.transpose(1, 0, 2, 3).reshape(COUT, 16, 32))
    return out

